# revision 15
# baseline (speedup 1.0000x reference)
"""Local (sliding-window) MQA attention block on 8 Trainium2 NeuronCores.

Sharding: data-parallel over batch (4) x sequence-parallel over query halves
(2) = 8 cores. Each core computes 1024 query rows of one batch against a
2048-row key halo (window=1024), all 16 query heads, with the single shared
KV head replicated. Outputs are disjoint row-slices of the final projection,
so no cross-core collectives are needed.

v4 structure (all PE operands bf16, f32 PSUM accumulation):
  - all weight matrices host-pre-permuted to [128, chunk, cols] layout so
    every weight DMA is a contiguous per-partition copy (the einops-style
    rearranged DMAs cost ~20us each in descriptor generation/transfer)
  - x^T resident in SBUF, loaded once over two HW DMA queues
  A  k/v projection: 8 interleaved accumulation chains, one continuous
     128-matmul burst; RoPE(k) in the drain; v transposed via XBAR DMA.
  B  q projection: chain-major matmul order (one PSUM bank at a time) with
     the rope drain of each chain emitted immediately after it.
  C  attention, software-pipelined ACROSS head-blocks: slots paired into
     2-bank PSUM tiles (one exp per pair), per-slot column ranges,
     halo-invalid slots cancelled via a denominator correction applied on
     the broadcast denominator; 1/den via reciprocal_approx_fast.
  D  output projection, weights prefetched during C into x^T's old space.
"""
import sys

for _p in ("/opt/trn_rl_repo",):
    if _p not in sys.path:
        sys.path.insert(0, _p)

import numpy as np
import ml_dtypes

import concourse.bass as bass
import concourse.bacc as bacc
import concourse.tile as tile
import concourse.mybir as mybir
from concourse.bass_utils import run_bass_kernel_spmd

F32 = mybir.dt.float32
BF16 = mybir.dt.bfloat16
EXP = mybir.ActivationFunctionType.Exp
BFNP = ml_dtypes.bfloat16

B, T, W = 4, 2048, 2048
NH, HD = 16, 128
WIN = 1024
QL = 1024          # query rows per core
KB = 2048          # key-halo rows per core
QBS = 512          # query block (moving free dim)
NQB = QL // QBS    # 2 query blocks per core
SLOTS = (WIN + QBS) // 128  # 12 key slots of 128 per query block
NW = W // 128      # 16 width chunks
NEG = -1.0e9
SCALE = HD ** -0.5
MAX_WAVELENGTH = 10000.0

# Valid query-column range per key slot (cols relative to the 512-query
# block). Slot k covers halo keys [512*i + 128*k - 1024, +128); columns
# outside the range are fully masked by window/causality and are simply not
# computed. Ranges for slots 0/11 widened so exp pairs have equal widths.
RANGES = [(0, 256), (0, 256), (0, 384), (0, 512),
          (0, 512), (0, 512), (0, 512), (0, 512),
          (0, 512), (128, 512), (256, 512), (256, 512)]
# Sub-range of RANGES needing an additive mask (staircase triangles plus the
# fully-invalid widened parts of slots 0/11).
MASKR = {0: (0, 256), 1: (128, 256), 2: (256, 384), 3: (384, 512),
         8: (0, 128), 9: (128, 256), 10: (256, 384), 11: (256, 512)}
MOFF = {}
_off = 0
for _k in sorted(MASKR):
    MOFF[_k] = _off
    _off += MASKR[_k][1] - MASKR[_k][0]
MTOT = _off  # 1280
# Slot pairs of equal range width; each pair shares one 2-bank PSUM tile and
# one exp. Slot data is stored bank-local at offset 0 (shifted by r0).
PAIRS = [(4, 5), (6, 7), (3, 8), (0, 1), (2, 9), (10, 11)]
POOL_DEN = {0, 1}  # pair indices whose denominator sums run on gpsimd

_COMPILED = None


def _rope_tables(pos):
    """pos: [n] int -> packed [128, n] f32: rows 0:64 = cos multipliers for
    dims 0:64, rows 64:128 = sin multipliers (negated for dims 0:32), such
    that rope(x)[d] = x[d]*cs[d] + x[d^32]*cs[64+d] for d<64; pass-through
    for d>=64."""
    half = 32
    inv_freq = MAX_WAVELENGTH ** (-(2.0 * np.arange(half, dtype=np.float64)) / 64.0)
    ang = pos.astype(np.float64)[None, :] * inv_freq[:, None]   # [32, n]
    sin, cos = np.sin(ang), np.cos(ang)
    n = pos.shape[0]
    cs = np.zeros((HD, n), dtype=np.float64)
    cs[0:32] = cos
    cs[32:64] = cos
    cs[64:96] = -sin
    cs[96:128] = sin
    return cs.astype(np.float32)


def _emit_rope(nc, pool, dst, src_ps, cs, n):
    """dst[0:64] = src[0:64]*cs[0:64] + shuf(src)[0:64]*cs[64:128];
    dst[64:128] = src[64:128]. dst: SBUF bf16 AP [128, n]; src_ps: PSUM f32
    AP [128, n]; cs: packed rope table SBUF f32 [128, n].
    DVE requires equal SBUF base partitions for both inputs, so the temps
    live in partitions 64:128 to match cs's sin half; PSUM inputs are
    exempt from the rule."""
    t1 = pool.tile([128, n], F32, tag="rope_t1", bufs=2)
    t2 = pool.tile([128, n], F32, tag="rope_t2", bufs=2)
    stage = pool.tile([128, n], F32, tag="rope_stage", bufs=2)
    nc.vector.tensor_mul(t1[64:128, :], src_ps[0:64, :], cs[0:64, :])
    nc.scalar.copy(out=stage[64:96, :], in_=src_ps[32:64, :])
    nc.scalar.copy(out=stage[96:128, :], in_=src_ps[0:32, :])
    nc.vector.tensor_mul(t2[64:128, :], stage[64:128, :], cs[64:128, :])
    nc.vector.tensor_add(dst[0:64, :], t1[64:128, :], t2[64:128, :])
    nc.scalar.copy(out=dst[64:128, :], in_=src_ps[64:128, :])


def _build_program():
    nc = bacc.Bacc("TRN2", target_bir_lowering=False, debug=False)

    xt = nc.dram_tensor("xt", [W, KB], BF16, kind="ExternalInput")
    # host-pre-permuted weights: straight [128, ...] per-partition layouts
    wqt = nc.dram_tensor("wqt", [128, 4, NW * 512], BF16, kind="ExternalInput")
    wkt = nc.dram_tensor("wkt", [128, NW * HD], BF16, kind="ExternalInput")
    wvt = nc.dram_tensor("wvt", [128, NW * HD], BF16, kind="ExternalInput")
    wot = nc.dram_tensor("wot", [128, 4, NW * 512], BF16, kind="ExternalInput")
    bias = nc.dram_tensor("bias", [W], F32, kind="ExternalInput")
    csq_d = nc.dram_tensor("csq", [HD, QL], F32, kind="ExternalInput")
    csk_d = nc.dram_tensor("csk", [HD, KB], F32, kind="ExternalInput")
    maskp_d = nc.dram_tensor("maskp", [NQB, 128, MTOT], BF16,
                             kind="ExternalInput")
    dcor_d = nc.dram_tensor("dencorr", [NQB, 128, QBS], F32,
                            kind="ExternalInput")
    ident_d = nc.dram_tensor("ident", [128, 128], BF16, kind="ExternalInput")
    ones_d = nc.dram_tensor("ones", [128, 1], BF16, kind="ExternalInput")
    out = nc.dram_tensor("out", [QL, W], F32, kind="ExternalOutput")

    with tile.TileContext(nc) as tc:
        with tc.tile_pool(name="persist", bufs=1) as pp, \
             tc.tile_pool(name="qpool", bufs=1) as qp:
            encT = pp.tile([HD, NH, QL], BF16, tag="encT")  # per-head enc^T
            kT_sb = pp.tile([HD, KB], BF16, tag="kT")       # rope'd k^T
            v_sb = pp.tile([128, KB], BF16, tag="v")        # natural v
            ones_sb = pp.tile([128, 1], BF16, tag="ones")
            ident = pp.tile([128, 128], BF16, tag="ident")
            masks = pp.tile([128, NQB, MTOT], BF16, tag="masks")
            dcor = pp.tile([128, NQB, QBS], F32, tag="dcor")
            qT = [qp.tile([HD, 8, QL], BF16, tag=f"qT{g}", name=f"qT{g}")
                  for g in range(2)]

            xtp = tc.tile_pool(name="xtp", bufs=1)
            xtp_ = xtp.__enter__()
            xt_sb = xtp_.tile([128, NW, KB], BF16, tag="xt")
            wqp = tc.tile_pool(name="wqpool", bufs=2)
            wqp_ = wqp.__enter__()

            # ---- prefetch: x^T chunks split across the sync and scalar HW
            # DMA queues (even/odd); small tables/masks on gpsimd ----
            def issue_xt(wc):
                q = nc.sync if wc % 2 == 0 else nc.scalar
                q.dma_start(out=xt_sb[:, wc, :],
                            in_=xt[128 * wc:128 * (wc + 1), :])

            nc.gpsimd.dma_start(out=ones_sb[:, :], in_=ones_d[:, :])
            nc.gpsimd.dma_start(out=ident[:, :], in_=ident_d[:, :])

            def issue_wq(widx):
                t = wqp_.tile([128, NW, 512], BF16, tag="wqw")
                nc.scalar.dma_start(out=t[:, :, :], in_=wqt[:, widx, :])
                return t

            # ---------- Phases A+B share one PSUM pool: per-tile WAR
            # instead of a pool-close arena barrier ----------
            abps = tc.tile_pool(name="pab_ps", bufs=1, space="PSUM")
            abps_ = abps.__enter__()

            def ab_tile(n):
                return abps_.tile([HD, 512], F32, tag=f"ab{n}",
                                  name=f"ab{n}")

            # ---------- Phase A: k/v projections over the halo ----------
            with nc.named_scope("phaseA"), \
                 tc.tile_pool(name="pa", bufs=2) as pa, \
                 tc.tile_pool(name="pa1", bufs=1) as pa1:
                wk_sb = pa1.tile([128, NW, HD], BF16, tag="wk")
                wv_sb = pa1.tile([128, NW, HD], BF16, tag="wv")
                nc.sync.dma_start(out=wk_sb[:, :, :], in_=wkt[:, :])
                nc.scalar.dma_start(out=wv_sb[:, :, :], in_=wvt[:, :])
                for wc in range(NW):
                    issue_xt(wc)
                wq_tiles = [issue_wq(0), issue_wq(1)]
                csk = pa1.tile([HD, KB], F32, tag="csk")
                nc.gpsimd.dma_start(out=csk[:, :], in_=csk_d[:, :])
                # 8 interleaved chains: (k/v) x halo quarter; tiles are
                # ordered so phase B's chains reuse them in free-order
                kt_ps = [ab_tile(2 * q4) for q4 in range(4)]
                vt_ps = [ab_tile(2 * q4 + 1) for q4 in range(4)]
                for wc in range(NW):
                    for q4 in range(4):
                        nc.tensor.matmul(
                            out=kt_ps[q4][:, :], lhsT=wk_sb[:, wc, :],
                            rhs=xt_sb[:, wc, 512 * q4:512 * (q4 + 1)],
                            start=(wc == 0), stop=(wc == NW - 1))
                    for q4 in range(4):
                        nc.tensor.matmul(
                            out=vt_ps[q4][:, :], lhsT=wv_sb[:, wc, :],
                            rhs=xt_sb[:, wc, 512 * q4:512 * (q4 + 1)],
                            start=(wc == 0), stop=(wc == NW - 1))
                for q4 in range(4):
                    cols = slice(512 * q4, 512 * (q4 + 1))
                    _emit_rope(nc, pa, kT_sb[:, cols], kt_ps[q4][:, :],
                               csk[:, cols], 512)
                    vt_sb = pa.tile([HD, 512], BF16, tag="vt_sb", bufs=2)
                    nc.vector.tensor_copy(out=vt_sb[:, :],
                                          in_=vt_ps[q4][:, :])
                    for j in range(4):
                        blk = 4 * q4 + j
                        nc.sync.dma_start(
                            out=v_sb[:, 128 * blk:128 * (blk + 1)],
                            in_=vt_sb[:, 128 * j:128 * (j + 1)],
                            transpose=True)

            # ---------- Phase B: q projection (chain-major) ----------
            with nc.named_scope("phaseB"), \
                 tc.tile_pool(name="pb", bufs=2) as pb:
                csq = pb.tile([HD, QL], F32, tag="csq", bufs=1)
                nc.gpsimd.dma_start(out=csq[:, :], in_=csq_d[:, :])
                for widx in range(4):
                    grp, wave = widx // 2, widx % 2
                    wq_w = wq_tiles[widx]
                    for j4 in range(4):
                        n8 = wave * 4 + j4
                        for qh in range(2):
                            q_ps = ab_tile(2 * j4 + qh)
                            for wc in range(NW):
                                nc.tensor.matmul(
                                    out=q_ps[:, :],
                                    lhsT=wq_w[:, wc,
                                              128 * j4:128 * (j4 + 1)],
                                    rhs=xt_sb[:, wc,
                                              WIN + QBS * qh:
                                              WIN + QBS * (qh + 1)],
                                    start=(wc == 0), stop=(wc == NW - 1))
                            _emit_rope(
                                nc, pb,
                                qT[grp][:, n8, QBS * qh:QBS * (qh + 1)],
                                q_ps[:, :],
                                csq[:, QBS * qh:QBS * (qh + 1)], QBS)
                    if widx + 2 < 4:
                        wq_tiles.append(issue_wq(widx + 2))

            abps.__exit__(None, None, None)
            wqp.__exit__(None, None, None)
            xtp.__exit__(None, None, None)

            # wot + bias prefetch (lands in the address space xt_sb vacated)
            with tc.tile_pool(name="pd", bufs=1) as pd:
                wot_sb = [pd.tile([128, NW, 512], BF16, tag=f"wot{oc}",
                                  name=f"wot{oc}") for oc in range(4)]
                bias_bc = pd.tile([128, W], F32, tag="biasbc")
                for oc in range(4):
                    qeng = nc.scalar if oc % 2 else nc.gpsimd
                    qeng.dma_start(out=wot_sb[oc][:, :, :],
                                   in_=wot[:, oc, :])
                nc.gpsimd.dma_start(out=masks[:, :, :], in_=maskp_d.ap()
                                    .rearrange("i p m -> p i m"))
                nc.gpsimd.dma_start(out=dcor[:, :, :], in_=dcor_d.ap()
                                    .rearrange("i p m -> p i m"))
                b_ap = bias.ap()
                nc.gpsimd.dma_start(out=bias_bc[:, :], in_=bass.AP(
                    tensor=b_ap.tensor, offset=b_ap.offset,
                    ap=[[0, 128]] + list(b_ap.ap)))

                # ------ Phase C: attention, pipelined across head-blocks ---
                with nc.named_scope("phaseC"), \
                     tc.tile_pool(name="pc", bufs=2) as pc, \
                     tc.tile_pool(name="et", bufs=4) as pe_t, \
                     tc.tile_pool(name="pc_s", bufs=2, space="PSUM") as pcs, \
                     tc.tile_pool(name="pc_a", bufs=2, space="PSUM") as pca:
                    BLOCKS = [(grp, i, n8) for grp in range(2)
                              for i in range(NQB) for n8 in range(8)]
                    NG = len(PAIRS)
                    ctx = {}   # b_idx -> (enc_ps, den_ps, {g: et2})

                    def emit_s(b, g):
                        grp, i, n8 = BLOCKS[b]
                        ka, kb_ = PAIRS[g]
                        wpr = RANGES[ka][1] - RANGES[ka][0]
                        s2 = pcs.tile([128, 2, QBS], F32, tag="s2")
                        for sub, k in ((0, ka), (1, kb_)):
                            r0, r1 = RANGES[k]
                            c0 = 512 * i + 128 * k
                            msk = k in MASKR
                            nc.tensor.matmul(
                                out=s2[:, sub, 0:r1 - r0],
                                lhsT=kT_sb[:, c0:c0 + 128],
                                rhs=qT[grp][:, n8,
                                            QBS * i + r0:QBS * i + r1],
                                start=True, stop=not msk,
                                skip_group_check=True)
                            if msk:
                                m0, m1 = MASKR[k]
                                nc.tensor.matmul(
                                    out=s2[:, sub, m0 - r0:m1 - r0],
                                    lhsT=ident[:, :],
                                    rhs=masks[:, i,
                                              MOFF[k]:MOFF[k] + (m1 - m0)],
                                    start=False, stop=True,
                                    skip_group_check=True)
                        et2 = pe_t.tile([128, 2, QBS], BF16, tag="et")
                        nc.scalar.activation(out=et2[:, :, 0:wpr],
                                             in_=s2[:, :, 0:wpr], func=EXP)
                        ctx[b][2][g] = et2
                        if g in POOL_DEN:
                            ar = pc.tile([128, 2, QBS], F32, tag=f"ar{g}",
                                         name=f"ar{g}")
                            import concourse.bass_isa as bass_isa
                            nc.gpsimd.partition_all_reduce(
                                out_ap=ar[:, :, :], in_ap=et2[:, :, :],
                                channels=128,
                                reduce_op=bass_isa.ReduceOp.add)
                            ctx[b][3][g] = ar

                    def emit_acc(b, g):
                        grp, i, n8 = BLOCKS[b]
                        enc_ps, den_ps, ets, ars, tar = ctx[b]
                        ka, kb_ = PAIRS[g]
                        last = (g == NG - 1)
                        if g in POOL_DEN:
                            ar = ars[g]
                            if g == 0:
                                nc.vector.tensor_add(
                                    tar[:, :], ar[:, 0, :], ar[:, 1, :])
                            else:
                                nc.vector.tensor_add(
                                    tar[:, :], tar[:, :], ar[:, 0, :])
                                nc.vector.tensor_add(
                                    tar[:, :], tar[:, :], ar[:, 1, :])
                        else:
                            for sub, k in ((0, ka), (1, kb_)):
                                r0, r1 = RANGES[k]
                                nc.tensor.matmul(
                                    out=den_ps[:, r0:r1],
                                    lhsT=ones_sb[:, :],
                                    rhs=ets[g][:, sub, 0:r1 - r0],
                                    start=(g == 2 and sub == 0),
                                    stop=last and (sub == 1),
                                    skip_group_check=True)
                        for sub, k in ((0, ka), (1, kb_)):
                            r0, r1 = RANGES[k]
                            blk = 4 * i + k
                            nc.tensor.matmul(
                                out=enc_ps[:, r0:r1],
                                lhsT=v_sb[:, 128 * blk:128 * (blk + 1)],
                                rhs=ets[g][:, sub, 0:r1 - r0],
                                start=(g == 0 and sub == 0),
                                stop=last and (sub == 1),
                                skip_group_check=True)
                        if last:
                            emit_norm(b)

                    def emit_norm(b):
                        grp, i, n8 = BLOCKS[b]
                        enc_ps, den_ps, _, _, tar = ctx[b]
                        head = grp * 8 + n8
                        den_s = pc.tile([1, QBS], F32, tag="den_s")
                        nc.vector.tensor_copy(out=den_s[:, :],
                                              in_=den_ps[:, :])
                        den_bc = pc.tile([128, QBS], F32, tag="den_bc")
                        nc.gpsimd.partition_broadcast(den_bc[:, :],
                                                      den_s[:, :])
                        nc.vector.tensor_add(tar[:, :], tar[:, :],
                                             dcor[:, i, :])
                        nc.vector.tensor_add(tar[:, :], tar[:, :],
                                             den_bc[:, :])
                        den_rc = pc.tile([128, QBS], F32, tag="den_rc")
                        nc.vector.reciprocal_approx_fast(out=den_rc[:, :],
                                                         in_=tar[:, :])
                        nc.vector.tensor_mul(
                            encT[:, head, QBS * i:QBS * (i + 1)],
                            enc_ps[:, :], den_rc[:, :])
                        del ctx[b]

                    tasks = [(b, g) for b in range(len(BLOCKS))
                             for g in range(NG)]
                    LA = 3
                    for idx, (b, g) in enumerate(tasks):
                        if g == 0:
                            enc_ps = pca.tile([HD, QBS], F32, tag="enc_ps")
                            den_ps = pca.tile([1, QBS], F32, tag="den_ps")
                            tar = pc.tile([128, QBS], F32, tag="tar")
                            ctx[b] = (enc_ps, den_ps, {}, {}, tar)
                        emit_s(b, g)
                        if idx >= LA:
                            emit_acc(*tasks[idx - LA])
                    for idx in range(len(tasks) - LA, len(tasks)):
                        emit_acc(*tasks[idx])

                # ---------- Phase D: output projection ----------
                with nc.named_scope("phaseD"), \
                     tc.tile_pool(name="pdo", bufs=3) as pdo, \
                     tc.tile_pool(name="pd_ps", bufs=3, space="PSUM") as pdps:
                    for oc in range(4):
                        for tsub in range(QL // 128):
                            o_ps = pdps.tile([128, 512], F32, tag="o_ps")
                            for n in range(NH):
                                nc.tensor.matmul(
                                    out=o_ps[:, :],
                                    lhsT=encT[:, n,
                                              128 * tsub:128 * (tsub + 1)],
                                    rhs=wot_sb[oc][:, n, :],
                                    start=(n == 0), stop=(n == NH - 1))
                            o_sb = pdo.tile([128, 512], F32, tag="o_sb")
                            nc.vector.tensor_add(
                                o_sb[:, :], o_ps[:, :],
                                bias_bc[:, 512 * oc:512 * (oc + 1)])
                            nc.sync.dma_start(
                                out=out[128 * tsub:128 * (tsub + 1),
                                        512 * oc:512 * (oc + 1)],
                                in_=o_sb[:, :])

    nc.compile()
    return nc


def _get_program():
    global _COMPILED
    if _COMPILED is None:
        _COMPILED = _build_program()
    return _COMPILED


def _perm_w(wt):
    """[W, C] (row-major x-dim) -> [128, NW*C]: row c*128+p col j -> p, c*C+j"""
    C = wt.shape[1]
    return np.ascontiguousarray(
        wt.reshape(NW, 128, C).transpose(1, 0, 2).reshape(128, NW * C))


def _prep_core_inputs(x, segment_pos, attention_mask, shared):
    """Per-core input dicts. Core c: batch c//2, query half c%2."""
    segment_pos = np.asarray(segment_pos)
    attention_mask = np.asarray(attention_mask)
    in_maps = []
    for c in range(8):
        b, h = c // 2, c % 2
        key_start = QL * h - WIN
        # halo buffer rows [key_start, key_start + KB) of batch b, zero-padded
        kb = np.zeros((KB, W), dtype=np.float32)
        lo = max(0, -key_start)
        kb[lo:] = x[b, key_start + lo:key_start + KB]
        xt = np.ascontiguousarray(kb.T).astype(BFNP)

        g_q = QL * h + np.arange(QL)                      # global query rows
        g_k = key_start + np.arange(KB)                   # global key rows
        pos_q = segment_pos[g_q]
        pos_k = np.where((g_k >= 0) & (g_k < T), segment_pos[np.clip(g_k, 0, T - 1)], 0)
        csq = _rope_tables(pos_q)
        csk = _rope_tables(pos_k)

        # Additive mask per (query block i, slot k) in S^T layout [ds, dt].
        # Halo-invalid slots are whole-slot; they get NO penalty here (their
        # exp(0)=1 den contribution is cancelled via dencorr; v rows are 0).
        ma = np.zeros((NQB, SLOTS, 128, QBS), dtype=np.float32)
        inv = np.zeros((NQB, SLOTS), dtype=bool)
        for i in range(NQB):
            t_glob = g_q[QBS * i:QBS * (i + 1)]           # [dt=512]
            for k in range(SLOTS):
                r = QBS * i + 128 * k + np.arange(128)    # halo rows [ds]
                s_glob = key_start + r
                ok = (s_glob >= 0) & (s_glob < T)
                if not ok.any():
                    inv[i, k] = True
                    continue
                if not ok.all():
                    raise ValueError("partial halo-invalid slot (unexpected)")
                m = attention_mask[t_glob[None, :].repeat(128, 0),
                                   s_glob[:, None]]
                ma[i, k][~m] = NEG
        if ma[:, 4:8].any():
            raise ValueError(
                "attention_mask penalizes interior window slots; this "
                "kernel assumes slots 4-7 are mask-free")
        # Validate the static range structure and pack the masks.
        maskp = np.zeros((NQB, 128, MTOT), dtype=np.float32)
        dencorr = np.zeros((NQB, 128, QBS), dtype=np.float32)
        for i in range(NQB):
            for k in range(SLOTS):
                r0, r1 = RANGES[k]
                if inv[i, k]:
                    dencorr[i, :, r0:r1] -= 128.0
                    continue
                outside = np.ones(QBS, dtype=bool)
                outside[r0:r1] = False
                if not (ma[i, k][:, outside] == NEG).all():
                    raise ValueError(
                        f"mask has unmasked entries outside the static "
                        f"range of slot {k}")
                if k in MASKR:
                    m0, m1 = MASKR[k]
                    inner = np.zeros(QBS, dtype=bool)
                    inner[r0:r1] = True
                    inner[m0:m1] = False
                    if ma[i, k][:, inner].any():
                        raise ValueError(
                            f"mask penalizes columns outside the static "
                            f"triangle of slot {k}")
                    maskp[i, :, MOFF[k]:MOFF[k] + (m1 - m0)] = \
                        ma[i, k][:, m0:m1]
                else:
                    if ma[i, k][:, r0:r1].any():
                        raise ValueError(f"mask penalizes interior slot {k}")
        in_maps.append(dict(shared, xt=xt, csq=csq, csk=csk,
                            maskp=maskp.astype(BFNP), dencorr=dencorr))
    return in_maps


def _check_mask_coverage(attention_mask):
    """Every True entry for core-c queries must fall inside its 12 slots."""
    am = np.asarray(attention_mask)
    t = np.arange(T)[:, None]
    s = np.arange(T)[None, :]
    h = (t >= QL).astype(np.int64)
    key_start = QL * h - WIN
    i = ((t - QL * h) // QBS)
    lo = key_start + QBS * i
    covered = (s >= lo) & (s < lo + SLOTS * 128)
    if (am & ~covered).any():
        raise ValueError(
            "attention_mask has True entries outside the sliding-window "
            "block structure this kernel is specialized for")


def kernel(x, segment_pos, attention_mask, wq, wk, wv, w_out, b_out):
    x = np.asarray(x, dtype=np.float32)
    wq = np.asarray(wq, dtype=np.float32)
    wk = np.asarray(wk, dtype=np.float32)
    wv = np.asarray(wv, dtype=np.float32)
    w_out = np.asarray(w_out, dtype=np.float32)
    b_out = np.asarray(b_out, dtype=np.float32)

    _check_mask_coverage(attention_mask)

    nc = _get_program()
    wq_p = _perm_w(np.ascontiguousarray(wq.T) * np.float32(SCALE))  # [128, NW*W]
    wot_p = _perm_w(np.ascontiguousarray(w_out.T))
    # regroup cols into the 4 head-group waves / out-col quarters
    wq_p = np.ascontiguousarray(
        wq_p.reshape(128, NW, W).transpose(0, 2, 1).reshape(128, 4, 512, NW)
        .transpose(0, 1, 3, 2).reshape(128, 4, NW * 512))
    wot_p = np.ascontiguousarray(
        wot_p.reshape(128, NW, W).transpose(0, 2, 1).reshape(128, 4, 512, NW)
        .transpose(0, 1, 3, 2).reshape(128, 4, NW * 512))
    shared = {
        "wqt": wq_p.astype(BFNP),
        "wkt": _perm_w(np.ascontiguousarray(wk.T)).astype(BFNP),
        "wvt": _perm_w(np.ascontiguousarray(wv.T)).astype(BFNP),
        "wot": wot_p.astype(BFNP),
        "bias": b_out,
        "ident": np.eye(128, dtype=np.float32).astype(BFNP),
        "ones": np.ones((128, 1), dtype=np.float32).astype(BFNP),
    }
    in_maps = _prep_core_inputs(x, segment_pos, attention_mask, shared)
    res = run_bass_kernel_spmd(nc, in_maps, list(range(8)))
    global _LAST_RESULT
    _LAST_RESULT = res

    out = np.empty((B, T, W), dtype=np.float32)
    for c in range(8):
        b, h = c // 2, c % 2
        out[b, QL * h:QL * (h + 1), :] = res.results[c]["out"]
    return out


# revision 16
# speedup vs baseline: 1.4736x; 1.4736x over previous
"""Local (sliding-window) MQA attention block on 8 Trainium2 NeuronCores.

Sharding: data-parallel over batch (4) x sequence-parallel over query halves
(2) = 8 cores. Each core computes 1024 query rows of one batch against a
2048-row key halo (window=1024), all 16 query heads, with the single shared
KV head replicated. Outputs are disjoint row-slices of the final projection,
so no cross-core collectives are needed.

v4 structure (all PE operands bf16, f32 PSUM accumulation):
  - all weight matrices host-pre-permuted to [128, chunk, cols] layout so
    every weight DMA is a contiguous per-partition copy (the einops-style
    rearranged DMAs cost ~20us each in descriptor generation/transfer)
  - x^T resident in SBUF, loaded once over two HW DMA queues
  A  k/v projection: 8 interleaved accumulation chains, one continuous
     128-matmul burst; RoPE(k) in the drain; v transposed via XBAR DMA.
  B  q projection: chain-major matmul order (one PSUM bank at a time) with
     the rope drain of each chain emitted immediately after it.
  C  attention, software-pipelined ACROSS head-blocks: slots paired into
     2-bank PSUM tiles (one exp per pair), per-slot column ranges,
     halo-invalid slots cancelled via a denominator correction applied on
     the broadcast denominator; 1/den via reciprocal_approx_fast.
  D  output projection, weights prefetched during C into x^T's old space.
"""
import sys

for _p in ("/opt/trn_rl_repo",):
    if _p not in sys.path:
        sys.path.insert(0, _p)

import numpy as np
import ml_dtypes

import concourse.bass as bass
import concourse.bacc as bacc
import concourse.tile as tile
import concourse.mybir as mybir
from concourse.bass_utils import run_bass_kernel_spmd

F32 = mybir.dt.float32
BF16 = mybir.dt.bfloat16
EXP = mybir.ActivationFunctionType.Exp
BFNP = ml_dtypes.bfloat16

B, T, W = 4, 2048, 2048
NH, HD = 16, 128
WIN = 1024
QL = 1024          # query rows per core
KB = 2048          # key-halo rows per core
QBS = 512          # query block (moving free dim)
NQB = QL // QBS    # 2 query blocks per core
SLOTS = (WIN + QBS) // 128  # 12 key slots of 128 per query block
NW = W // 128      # 16 width chunks
NEG = -1.0e9
SCALE = HD ** -0.5
MAX_WAVELENGTH = 10000.0

# Valid query-column range per key slot (cols relative to the 512-query
# block). Slot k covers halo keys [512*i + 128*k - 1024, +128); columns
# outside the range are fully masked by window/causality and are simply not
# computed. Ranges for slots 0/11 widened so exp pairs have equal widths.
RANGES = [(0, 256), (0, 256), (0, 384), (0, 512),
          (0, 512), (0, 512), (0, 512), (0, 512),
          (0, 512), (128, 512), (256, 512), (256, 512)]
# Sub-range of RANGES needing an additive mask (staircase triangles plus the
# fully-invalid widened parts of slots 0/11).
MASKR = {0: (0, 256), 1: (128, 256), 2: (256, 384), 3: (384, 512),
         8: (0, 128), 9: (128, 256), 10: (256, 384), 11: (256, 512)}
MOFF = {}
_off = 0
for _k in sorted(MASKR):
    MOFF[_k] = _off
    _off += MASKR[_k][1] - MASKR[_k][0]
MTOT = _off  # 1280
# Slot pairs of equal range width; each pair shares one 2-bank PSUM tile and
# one exp. Slot data is stored bank-local at offset 0 (shifted by r0).
PAIRS = [(3, 8), (0, 1), (2, 9), (10, 11), (4, 5), (6, 7)]

_COMPILED = None


def _rope_tables(pos):
    """pos: [n] int -> packed [128, n] f32: rows 0:64 = cos multipliers for
    dims 0:64, rows 64:128 = sin multipliers (negated for dims 0:32), such
    that rope(x)[d] = x[d]*cs[d] + x[d^32]*cs[64+d] for d<64; pass-through
    for d>=64."""
    half = 32
    inv_freq = MAX_WAVELENGTH ** (-(2.0 * np.arange(half, dtype=np.float64)) / 64.0)
    ang = pos.astype(np.float64)[None, :] * inv_freq[:, None]   # [32, n]
    sin, cos = np.sin(ang), np.cos(ang)
    n = pos.shape[0]
    cs = np.zeros((HD, n), dtype=np.float64)
    cs[0:32] = cos
    cs[32:64] = cos
    cs[64:96] = -sin
    cs[96:128] = sin
    return cs.astype(np.float32)


def _emit_rope(nc, pool, dst, src_ps, cs, n):
    """dst[0:64] = src[0:64]*cs[0:64] + shuf(src)[0:64]*cs[64:128];
    dst[64:128] = src[64:128]. dst: SBUF bf16 AP [128, n]; src_ps: PSUM f32
    AP [128, n]; cs: packed rope table SBUF f32 [128, n].
    DVE requires equal SBUF base partitions for both inputs, so the temps
    live in partitions 64:128 to match cs's sin half; PSUM inputs are
    exempt from the rule."""
    t1 = pool.tile([128, n], F32, tag="rope_t1", bufs=2)
    t2 = pool.tile([128, n], F32, tag="rope_t2", bufs=2)
    stage = pool.tile([128, n], F32, tag="rope_stage", bufs=2)
    nc.vector.tensor_mul(t1[64:128, :], src_ps[0:64, :], cs[0:64, :])
    nc.scalar.copy(out=stage[64:96, :], in_=src_ps[32:64, :])
    nc.scalar.copy(out=stage[96:128, :], in_=src_ps[0:32, :])
    nc.vector.tensor_mul(t2[64:128, :], stage[64:128, :], cs[64:128, :])
    nc.vector.tensor_add(dst[0:64, :], t1[64:128, :], t2[64:128, :])
    nc.scalar.copy(out=dst[64:128, :], in_=src_ps[64:128, :])


def _build_program():
    nc = bacc.Bacc("TRN2", target_bir_lowering=False, debug=False)

    xt = nc.dram_tensor("xt", [W, KB], BF16, kind="ExternalInput")
    # host-pre-permuted weights: straight [128, ...] per-partition layouts
    wqt = nc.dram_tensor("wqt", [128, 4, NW * 512], BF16, kind="ExternalInput")
    wkt = nc.dram_tensor("wkt", [128, NW * HD], BF16, kind="ExternalInput")
    wvt = nc.dram_tensor("wvt", [128, NW * HD], BF16, kind="ExternalInput")
    wot = nc.dram_tensor("wot", [128, 4, NW * 512], BF16, kind="ExternalInput")
    bias = nc.dram_tensor("bias", [W], F32, kind="ExternalInput")
    csq_d = nc.dram_tensor("csq", [HD, QL], F32, kind="ExternalInput")
    csk_d = nc.dram_tensor("csk", [HD, KB], F32, kind="ExternalInput")
    maskp_d = nc.dram_tensor("maskp", [NQB, 128, MTOT], BF16,
                             kind="ExternalInput")
    dcor_d = nc.dram_tensor("dencorr", [NQB, 128, QBS], F32,
                            kind="ExternalInput")
    ident_d = nc.dram_tensor("ident", [128, 128], BF16, kind="ExternalInput")
    ones_d = nc.dram_tensor("ones", [128, 1], BF16, kind="ExternalInput")
    out = nc.dram_tensor("out", [QL, W], F32, kind="ExternalOutput")

    with tile.TileContext(nc) as tc:
        with tc.tile_pool(name="persist", bufs=1) as pp, \
             tc.tile_pool(name="qpool", bufs=1) as qp:
            encT = pp.tile([HD, NH, QL], BF16, tag="encT")  # per-head enc^T
            kT_sb = pp.tile([HD, KB], BF16, tag="kT")       # rope'd k^T
            v_sb = pp.tile([128, KB], BF16, tag="v")        # natural v
            ones_sb = pp.tile([128, 1], BF16, tag="ones")
            ident = pp.tile([128, 128], BF16, tag="ident")
            masks = pp.tile([128, NQB, MTOT], BF16, tag="masks")
            dcor = pp.tile([128, NQB, QBS], F32, tag="dcor")
            qT = [qp.tile([HD, 8, QL], BF16, tag=f"qT{g}", name=f"qT{g}")
                  for g in range(2)]

            xtp = tc.tile_pool(name="xtp", bufs=1)
            xtp_ = xtp.__enter__()
            xt_sb = xtp_.tile([128, NW, KB], BF16, tag="xt")
            wqp = tc.tile_pool(name="wqpool", bufs=2)
            wqp_ = wqp.__enter__()

            # ---- prefetch: x^T chunks split across the sync and scalar HW
            # DMA queues (even/odd); small tables/masks on gpsimd ----
            def issue_xt(wc):
                q = nc.sync if wc % 2 == 0 else nc.scalar
                q.dma_start(out=xt_sb[:, wc, :],
                            in_=xt[128 * wc:128 * (wc + 1), :])

            nc.gpsimd.dma_start(out=ones_sb[:, :], in_=ones_d[:, :])
            nc.gpsimd.dma_start(out=ident[:, :], in_=ident_d[:, :])

            def issue_wq(widx):
                t = wqp_.tile([128, NW, 512], BF16, tag="wqw")
                nc.scalar.dma_start(out=t[:, :, :], in_=wqt[:, widx, :])
                return t

            # ---------- Phases A+B share one PSUM pool: per-tile WAR
            # instead of a pool-close arena barrier ----------
            abps = tc.tile_pool(name="pab_ps", bufs=1, space="PSUM")
            abps_ = abps.__enter__()

            def ab_tile(n):
                return abps_.tile([HD, 512], F32, tag=f"ab{n}",
                                  name=f"ab{n}")

            # ---------- Phase A: k/v projections over the halo ----------
            with nc.named_scope("phaseA"), \
                 tc.tile_pool(name="pa", bufs=2) as pa, \
                 tc.tile_pool(name="pa1", bufs=1) as pa1:
                wk_sb = pa1.tile([128, NW, HD], BF16, tag="wk")
                wv_sb = pa1.tile([128, NW, HD], BF16, tag="wv")
                nc.sync.dma_start(out=wk_sb[:, :, :], in_=wkt[:, :])
                nc.scalar.dma_start(out=wv_sb[:, :, :], in_=wvt[:, :])
                for wc in range(NW):
                    issue_xt(wc)
                wq_tiles = [issue_wq(0), issue_wq(1)]
                csk = pa1.tile([HD, KB], F32, tag="csk")
                nc.gpsimd.dma_start(out=csk[:, :], in_=csk_d[:, :])
                # 8 interleaved chains: (k/v) x halo quarter; tiles are
                # ordered so phase B's chains reuse them in free-order
                kt_ps = [ab_tile(2 * q4) for q4 in range(4)]
                vt_ps = [ab_tile(2 * q4 + 1) for q4 in range(4)]
                for wc in range(NW):
                    for q4 in range(4):
                        nc.tensor.matmul(
                            out=kt_ps[q4][:, :], lhsT=wk_sb[:, wc, :],
                            rhs=xt_sb[:, wc, 512 * q4:512 * (q4 + 1)],
                            start=(wc == 0), stop=(wc == NW - 1))
                    for q4 in range(4):
                        nc.tensor.matmul(
                            out=vt_ps[q4][:, :], lhsT=wv_sb[:, wc, :],
                            rhs=xt_sb[:, wc, 512 * q4:512 * (q4 + 1)],
                            start=(wc == 0), stop=(wc == NW - 1))
                for q4 in range(4):
                    cols = slice(512 * q4, 512 * (q4 + 1))
                    _emit_rope(nc, pa, kT_sb[:, cols], kt_ps[q4][:, :],
                               csk[:, cols], 512)
                    vt_sb = pa.tile([HD, 512], BF16, tag="vt_sb", bufs=2)
                    nc.vector.tensor_copy(out=vt_sb[:, :],
                                          in_=vt_ps[q4][:, :])
                    for j in range(4):
                        blk = 4 * q4 + j
                        nc.sync.dma_start(
                            out=v_sb[:, 128 * blk:128 * (blk + 1)],
                            in_=vt_sb[:, 128 * j:128 * (j + 1)],
                            transpose=True)

            # ---------- Phase B: q projection (chain-major) ----------
            with nc.named_scope("phaseB"), \
                 tc.tile_pool(name="pb", bufs=2) as pb:
                csq = pb.tile([HD, QL], F32, tag="csq", bufs=1)
                nc.gpsimd.dma_start(out=csq[:, :], in_=csq_d[:, :])
                for widx in range(4):
                    grp, wave = widx // 2, widx % 2
                    wq_w = wq_tiles[widx]
                    for j4 in range(4):
                        n8 = wave * 4 + j4
                        for qh in range(2):
                            q_ps = ab_tile(2 * j4 + qh)
                            for wc in range(NW):
                                nc.tensor.matmul(
                                    out=q_ps[:, :],
                                    lhsT=wq_w[:, wc,
                                              128 * j4:128 * (j4 + 1)],
                                    rhs=xt_sb[:, wc,
                                              WIN + QBS * qh:
                                              WIN + QBS * (qh + 1)],
                                    start=(wc == 0), stop=(wc == NW - 1))
                            _emit_rope(
                                nc, pb,
                                qT[grp][:, n8, QBS * qh:QBS * (qh + 1)],
                                q_ps[:, :],
                                csq[:, QBS * qh:QBS * (qh + 1)], QBS)
                    if widx + 2 < 4:
                        wq_tiles.append(issue_wq(widx + 2))

            abps.__exit__(None, None, None)
            wqp.__exit__(None, None, None)
            xtp.__exit__(None, None, None)

            # wot + bias prefetch (lands in the address space xt_sb vacated)
            with tc.tile_pool(name="pd", bufs=1) as pd:
                wot_sb = [pd.tile([128, NW, 512], BF16, tag=f"wot{oc}",
                                  name=f"wot{oc}") for oc in range(4)]
                bias_bc = pd.tile([128, W], F32, tag="biasbc")
                for oc in range(4):
                    qeng = nc.scalar if oc % 2 else nc.gpsimd
                    qeng.dma_start(out=wot_sb[oc][:, :, :],
                                   in_=wot[:, oc, :])
                nc.gpsimd.dma_start(out=masks[:, :, :], in_=maskp_d.ap()
                                    .rearrange("i p m -> p i m"))
                nc.gpsimd.dma_start(out=dcor[:, :, :], in_=dcor_d.ap()
                                    .rearrange("i p m -> p i m"))
                b_ap = bias.ap()
                nc.gpsimd.dma_start(out=bias_bc[:, :], in_=bass.AP(
                    tensor=b_ap.tensor, offset=b_ap.offset,
                    ap=[[0, 128]] + list(b_ap.ap)))

                # ------ Phase C: attention, pipelined across head-blocks ---
                with nc.named_scope("phaseC"), \
                     tc.tile_pool(name="pc", bufs=2) as pc, \
                     tc.tile_pool(name="et", bufs=4) as pe_t, \
                     tc.tile_pool(name="pc_s", bufs=2, space="PSUM") as pcs, \
                     tc.tile_pool(name="pc_a", bufs=2, space="PSUM") as pca:
                    BLOCKS = [(grp, i, n8) for grp in range(2)
                              for i in range(NQB) for n8 in range(8)]
                    NG = len(PAIRS)
                    ctx = {}   # b_idx -> (enc_ps, den_ps, {g: et2})

                    def emit_s(b, g):
                        grp, i, n8 = BLOCKS[b]
                        ka, kb_ = PAIRS[g]
                        wpr = RANGES[ka][1] - RANGES[ka][0]
                        s2 = pcs.tile([128, 2, QBS], F32, tag="s2")
                        for sub, k in ((0, ka), (1, kb_)):
                            r0, r1 = RANGES[k]
                            c0 = 512 * i + 128 * k
                            msk = k in MASKR
                            nc.tensor.matmul(
                                out=s2[:, sub, 0:r1 - r0],
                                lhsT=kT_sb[:, c0:c0 + 128],
                                rhs=qT[grp][:, n8,
                                            QBS * i + r0:QBS * i + r1],
                                start=True, stop=not msk,
                                skip_group_check=True)
                            if msk:
                                m0, m1 = MASKR[k]
                                nc.tensor.matmul(
                                    out=s2[:, sub, m0 - r0:m1 - r0],
                                    lhsT=ident[:, :],
                                    rhs=masks[:, i,
                                              MOFF[k]:MOFF[k] + (m1 - m0)],
                                    start=False, stop=True,
                                    skip_group_check=True)
                        et2 = pe_t.tile([128, 2, QBS], BF16, tag="et")
                        nc.scalar.activation(out=et2[:, :, 0:wpr],
                                             in_=s2[:, :, 0:wpr], func=EXP)
                        ctx[b][2][g] = et2

                    def emit_acc(b, g):
                        grp, i, n8 = BLOCKS[b]
                        enc_ps, den_ps, ets = ctx[b]
                        ka, kb_ = PAIRS[g]
                        last = (g == NG - 1)
                        for sub, k in ((0, ka), (1, kb_)):
                            r0, r1 = RANGES[k]
                            nc.tensor.matmul(
                                out=den_ps[:, r0:r1], lhsT=ones_sb[:, :],
                                rhs=ets[g][:, sub, 0:r1 - r0],
                                start=(g == 0 and sub == 0),
                                stop=last and (sub == 1),
                                skip_group_check=True)
                        for sub, k in ((0, ka), (1, kb_)):
                            r0, r1 = RANGES[k]
                            blk = 4 * i + k
                            nc.tensor.matmul(
                                out=enc_ps[:, r0:r1],
                                lhsT=v_sb[:, 128 * blk:128 * (blk + 1)],
                                rhs=ets[g][:, sub, 0:r1 - r0],
                                start=(g == 0 and sub == 0),
                                stop=last and (sub == 1),
                                skip_group_check=True)
                        if last:
                            emit_norm(b)

                    def emit_norm(b):
                        grp, i, n8 = BLOCKS[b]
                        enc_ps, den_ps, _ = ctx[b]
                        head = grp * 8 + n8
                        den_s = pc.tile([1, QBS], F32, tag="den_s")
                        nc.vector.tensor_copy(out=den_s[:, :],
                                              in_=den_ps[:, :])
                        den_bc = pc.tile([128, QBS], F32, tag="den_bc")
                        nc.gpsimd.partition_broadcast(den_bc[:, :],
                                                      den_s[:, :])
                        den_cr = pc.tile([128, QBS], F32, tag="den_cr")
                        nc.vector.tensor_add(den_cr[:, :], den_bc[:, :],
                                             dcor[:, i, :])
                        den_rc = pc.tile([128, QBS], F32, tag="den_rc")
                        nc.vector.reciprocal_approx_fast(out=den_rc[:, :],
                                                         in_=den_cr[:, :])
                        nc.vector.tensor_mul(
                            encT[:, head, QBS * i:QBS * (i + 1)],
                            enc_ps[:, :], den_rc[:, :])
                        del ctx[b]

                    tasks = [(b, g) for b in range(len(BLOCKS))
                             for g in range(NG)]
                    LA = 3
                    for idx, (b, g) in enumerate(tasks):
                        if g == 0:
                            enc_ps = pca.tile([HD, QBS], F32, tag="enc_ps")
                            den_ps = pca.tile([1, QBS], F32, tag="den_ps")
                            ctx[b] = (enc_ps, den_ps, {})
                        emit_s(b, g)
                        if idx >= LA:
                            emit_acc(*tasks[idx - LA])
                    for idx in range(len(tasks) - LA, len(tasks)):
                        emit_acc(*tasks[idx])

                # ---------- Phase D: output projection ----------
                with nc.named_scope("phaseD"), \
                     tc.tile_pool(name="pdo", bufs=3) as pdo, \
                     tc.tile_pool(name="pd_ps", bufs=3, space="PSUM") as pdps:
                    for oc in range(4):
                        for tsub in range(QL // 128):
                            o_ps = pdps.tile([128, 512], F32, tag="o_ps")
                            for n in range(NH):
                                nc.tensor.matmul(
                                    out=o_ps[:, :],
                                    lhsT=encT[:, n,
                                              128 * tsub:128 * (tsub + 1)],
                                    rhs=wot_sb[oc][:, n, :],
                                    start=(n == 0), stop=(n == NH - 1))
                            o_sb = pdo.tile([128, 512], F32, tag="o_sb")
                            nc.vector.tensor_add(
                                o_sb[:, :], o_ps[:, :],
                                bias_bc[:, 512 * oc:512 * (oc + 1)])
                            nc.sync.dma_start(
                                out=out[128 * tsub:128 * (tsub + 1),
                                        512 * oc:512 * (oc + 1)],
                                in_=o_sb[:, :])

    nc.compile()
    return nc


def _get_program():
    global _COMPILED
    if _COMPILED is None:
        _COMPILED = _build_program()
    return _COMPILED


def _perm_w(wt):
    """[W, C] (row-major x-dim) -> [128, NW*C]: row c*128+p col j -> p, c*C+j"""
    C = wt.shape[1]
    return np.ascontiguousarray(
        wt.reshape(NW, 128, C).transpose(1, 0, 2).reshape(128, NW * C))


def _prep_core_inputs(x, segment_pos, attention_mask, shared):
    """Per-core input dicts. Core c: batch c//2, query half c%2."""
    segment_pos = np.asarray(segment_pos)
    attention_mask = np.asarray(attention_mask)
    in_maps = []
    for c in range(8):
        b, h = c // 2, c % 2
        key_start = QL * h - WIN
        # halo buffer rows [key_start, key_start + KB) of batch b, zero-padded
        kb = np.zeros((KB, W), dtype=np.float32)
        lo = max(0, -key_start)
        kb[lo:] = x[b, key_start + lo:key_start + KB]
        xt = np.ascontiguousarray(kb.T).astype(BFNP)

        g_q = QL * h + np.arange(QL)                      # global query rows
        g_k = key_start + np.arange(KB)                   # global key rows
        pos_q = segment_pos[g_q]
        pos_k = np.where((g_k >= 0) & (g_k < T), segment_pos[np.clip(g_k, 0, T - 1)], 0)
        csq = _rope_tables(pos_q)
        csk = _rope_tables(pos_k)

        # Additive mask per (query block i, slot k) in S^T layout [ds, dt].
        # Halo-invalid slots are whole-slot; they get NO penalty here (their
        # exp(0)=1 den contribution is cancelled via dencorr; v rows are 0).
        ma = np.zeros((NQB, SLOTS, 128, QBS), dtype=np.float32)
        inv = np.zeros((NQB, SLOTS), dtype=bool)
        for i in range(NQB):
            t_glob = g_q[QBS * i:QBS * (i + 1)]           # [dt=512]
            for k in range(SLOTS):
                r = QBS * i + 128 * k + np.arange(128)    # halo rows [ds]
                s_glob = key_start + r
                ok = (s_glob >= 0) & (s_glob < T)
                if not ok.any():
                    inv[i, k] = True
                    continue
                if not ok.all():
                    raise ValueError("partial halo-invalid slot (unexpected)")
                m = attention_mask[t_glob[None, :].repeat(128, 0),
                                   s_glob[:, None]]
                ma[i, k][~m] = NEG
        if ma[:, 4:8].any():
            raise ValueError(
                "attention_mask penalizes interior window slots; this "
                "kernel assumes slots 4-7 are mask-free")
        # Validate the static range structure and pack the masks.
        maskp = np.zeros((NQB, 128, MTOT), dtype=np.float32)
        dencorr = np.zeros((NQB, 128, QBS), dtype=np.float32)
        for i in range(NQB):
            for k in range(SLOTS):
                r0, r1 = RANGES[k]
                if inv[i, k]:
                    dencorr[i, :, r0:r1] -= 128.0
                    continue
                outside = np.ones(QBS, dtype=bool)
                outside[r0:r1] = False
                if not (ma[i, k][:, outside] == NEG).all():
                    raise ValueError(
                        f"mask has unmasked entries outside the static "
                        f"range of slot {k}")
                if k in MASKR:
                    m0, m1 = MASKR[k]
                    inner = np.zeros(QBS, dtype=bool)
                    inner[r0:r1] = True
                    inner[m0:m1] = False
                    if ma[i, k][:, inner].any():
                        raise ValueError(
                            f"mask penalizes columns outside the static "
                            f"triangle of slot {k}")
                    maskp[i, :, MOFF[k]:MOFF[k] + (m1 - m0)] = \
                        ma[i, k][:, m0:m1]
                else:
                    if ma[i, k][:, r0:r1].any():
                        raise ValueError(f"mask penalizes interior slot {k}")
        in_maps.append(dict(shared, xt=xt, csq=csq, csk=csk,
                            maskp=maskp.astype(BFNP), dencorr=dencorr))
    return in_maps


def _check_mask_coverage(attention_mask):
    """Every True entry for core-c queries must fall inside its 12 slots."""
    am = np.asarray(attention_mask)
    t = np.arange(T)[:, None]
    s = np.arange(T)[None, :]
    h = (t >= QL).astype(np.int64)
    key_start = QL * h - WIN
    i = ((t - QL * h) // QBS)
    lo = key_start + QBS * i
    covered = (s >= lo) & (s < lo + SLOTS * 128)
    if (am & ~covered).any():
        raise ValueError(
            "attention_mask has True entries outside the sliding-window "
            "block structure this kernel is specialized for")


def kernel(x, segment_pos, attention_mask, wq, wk, wv, w_out, b_out):
    x = np.asarray(x, dtype=np.float32)
    wq = np.asarray(wq, dtype=np.float32)
    wk = np.asarray(wk, dtype=np.float32)
    wv = np.asarray(wv, dtype=np.float32)
    w_out = np.asarray(w_out, dtype=np.float32)
    b_out = np.asarray(b_out, dtype=np.float32)

    _check_mask_coverage(attention_mask)

    nc = _get_program()
    wq_p = _perm_w(np.ascontiguousarray(wq.T) * np.float32(SCALE))  # [128, NW*W]
    wot_p = _perm_w(np.ascontiguousarray(w_out.T))
    # regroup cols into the 4 head-group waves / out-col quarters
    wq_p = np.ascontiguousarray(
        wq_p.reshape(128, NW, W).transpose(0, 2, 1).reshape(128, 4, 512, NW)
        .transpose(0, 1, 3, 2).reshape(128, 4, NW * 512))
    wot_p = np.ascontiguousarray(
        wot_p.reshape(128, NW, W).transpose(0, 2, 1).reshape(128, 4, 512, NW)
        .transpose(0, 1, 3, 2).reshape(128, 4, NW * 512))
    shared = {
        "wqt": wq_p.astype(BFNP),
        "wkt": _perm_w(np.ascontiguousarray(wk.T)).astype(BFNP),
        "wvt": _perm_w(np.ascontiguousarray(wv.T)).astype(BFNP),
        "wot": wot_p.astype(BFNP),
        "bias": b_out,
        "ident": np.eye(128, dtype=np.float32).astype(BFNP),
        "ones": np.ones((128, 1), dtype=np.float32).astype(BFNP),
    }
    in_maps = _prep_core_inputs(x, segment_pos, attention_mask, shared)
    res = run_bass_kernel_spmd(nc, in_maps, list(range(8)))
    global _LAST_RESULT
    _LAST_RESULT = res

    out = np.empty((B, T, W), dtype=np.float32)
    for c in range(8):
        b, h = c // 2, c % 2
        out[b, QL * h:QL * (h + 1), :] = res.results[c]["out"]
    return out
